# revision 1
# baseline (speedup 1.0000x reference)
"""Causal single-head attention (B=4, S=4096, E=1024, D=128) on 8 TRN2 cores.

Sharding: core c = (batch b = c//2, half h = c%2). Each core computes the
output rows for queries [h*2048, (h+1)*2048) of batch b. Its key/value pool
is the whole sequence reordered as [own half | other half] so that every
core runs the *same* graph (SPMD): a causal diagonal chunk (first 2048 pool
keys) plus a full-attention rectangle chunk (last 2048 pool keys) whose
contribution is gated by a per-core additive bias (0 for h=1, -1e9 for h=0)
fused into the ScalarE exp. No collectives are needed.

The host passes x pre-transposed per core (xT [E, C] f32, a pure layout
shuffle). Projections consume the f32 tiles directly as float32r matmuls
(full TensorE rate at N=512), so x is never converted or re-staged on chip.

Compute layout: scores are built transposed ([k, q]) so the key axis lands
on partitions; the causal/key masks then fuse into the exp (bias / DVE add)
and the AV matmul consumes exp(scoresT) directly with V as the stationary
operand. Softmax skips max-subtraction (scores/32 stay in [-8, 8] for randn
inputs). Scores/AV matmuls run in bf16 (1 cycle/row); accumulation f32 PSUM.
K^T/Q^T/V live in per-512-token tiles so attention overlaps the projection
phase (Tile tracks deps per tile).
"""

import sys

if "/opt/trn_rl_repo" not in sys.path:
    sys.path.insert(0, "/opt/trn_rl_repo")

import numpy as np

B, S, E, D = 4, 4096, 1024, 128
H = S // 2  # queries per core
C = S  # pool keys per core
SCALE = 1.0 / 32.0  # 1/sqrt(E)
NEG = -1.0e9
P = 128  # partitions
QW = 512  # query group width
KB = 128  # key block
XW = 1024  # x DMA chunk width (tokens)


def _build(nc_args=None):
    import concourse.bass as bass  # noqa: F401
    import concourse.mybir as mybir
    import concourse.tile as tile
    from concourse import bacc
    from concourse.masks import make_identity

    f32 = mybir.dt.float32
    f32r = mybir.dt.float32r
    bf16 = mybir.dt.bfloat16

    nc = bacc.Bacc(
        "TRN2",
        target_bir_lowering=False,
        debug=False,
        enable_asserts=False,
        num_devices=8,
    )

    xt_d = nc.dram_tensor("xt", [E, C], f32, kind="ExternalInput").ap()
    wq_d = nc.dram_tensor("wq", [E, D], f32, kind="ExternalInput").ap()
    wk_d = nc.dram_tensor("wk", [E, D], f32, kind="ExternalInput").ap()
    wv_d = nc.dram_tensor("wv", [E, D], f32, kind="ExternalInput").ap()
    km_d = nc.dram_tensor("km", [P, (C - H) // KB], f32, kind="ExternalInput").ap()
    out_d = nc.dram_tensor("out", [H, D], f32, kind="ExternalOutput").ap()

    ECH = E // P  # e-chunks (8)
    NSG = C // QW  # s-groups of 512 over the pool (8)
    NQG = H // QW  # q-groups (4)
    RB0 = H // KB  # first rect k-block (16)
    NKB = C // KB  # total k-blocks (32)
    DIAG_PB = QW // KB  # partial-diag blocks per q-group (4)
    W2 = 2 * QW  # 1024: double-bank score tiles
    GPX = XW // QW  # s-groups per x chunk (2)

    with tile.TileContext(nc) as tc:
        from contextlib import ExitStack

        with ExitStack() as ctx:
            consts = ctx.enter_context(tc.tile_pool(name="consts", bufs=1))
            xraw_p = ctx.enter_context(tc.tile_pool(name="xraw", bufs=16))
            kv_p = ctx.enter_context(tc.tile_pool(name="kv", bufs=1))
            vtsb_p = ctx.enter_context(tc.tile_pool(name="vtsb", bufs=2))
            expt_p = ctx.enter_context(tc.tile_pool(name="expt", bufs=8))
            avn_p = ctx.enter_context(tc.tile_pool(name="avn", bufs=2))
            outsb_p = ctx.enter_context(tc.tile_pool(name="outsb", bufs=3))
            rec_p = ctx.enter_context(tc.tile_pool(name="rec", bufs=2))
            ps_sc = ctx.enter_context(tc.tile_pool(name="ps_sc", bufs=3, space="PSUM"))
            ps_proj = ctx.enter_context(
                tc.tile_pool(name="ps_proj", bufs=2, space="PSUM")
            )
            ps_small = ps_proj
            ps_av = ctx.enter_context(tc.tile_pool(name="ps_av", bufs=2, space="PSUM"))
            ps_den = ctx.enter_context(
                tc.tile_pool(name="ps_den", bufs=1, space="PSUM")
            )

            # ---- constants ----
            ident = consts.tile([P, P], bf16, tag="ident")
            make_identity(nc, ident[:])
            ones = consts.tile([P, 1], bf16, tag="ones")
            nc.gpsimd.memset(ones[:], 1.0)
            identf = consts.tile([1, 1], f32, tag="identf")
            nc.gpsimd.memset(identf[:], 1.0)
            # staircase masks: masks[:, i*QW:(i+1)*QW] has delta = i*KB;
            # mask[p, f] = 0 if p + delta <= f else NEG  (f = local q, p = local k)
            masks = consts.tile([P, DIAG_PB * QW], f32, tag="masks")
            nc.gpsimd.memset(masks[:], NEG)
            for i in range(DIAG_PB):
                nc.gpsimd.affine_select(
                    out=masks[:, i * QW : (i + 1) * QW],
                    in_=masks[:, i * QW : (i + 1) * QW],
                    compare_op=mybir.AluOpType.is_ge,
                    fill=0.0,
                    base=i * KB - 1,
                    pattern=[[-1, QW]],
                    channel_multiplier=1,
                )
            km_sb = consts.tile([P, RB0], f32, tag="km")
            nc.scalar.dma_start(km_sb[:], km_d[:])

            # ---- weights: one DMA each, [E, D] -> [P, ECH*D] (chunk ec at ec*D) ----
            w_sbufs = {}
            for name, w_d in (("wq", wq_d), ("wk", wk_d), ("wv", wv_d)):
                w_sb = consts.tile([P, ECH * D], f32r, tag=f"w_{name}", name=f"wsb_{name}")
                w_sbufs[name] = w_sb
                nc.scalar.dma_start(
                    w_sb[:].rearrange("p (ec d) -> p ec d", d=D),
                    w_d.rearrange("(ec p) d -> p ec d", p=P).bitcast(f32r),
                )
            wq_sb, wk_sb, wv_sb = w_sbufs["wq"], w_sbufs["wk"], w_sbufs["wv"]

            # per-s-group projected tiles (separate tiles -> fine-grained deps)
            kt_g = [
                kv_p.tile([P, QW], bf16, tag=f"kt{g}", name=f"kt{g}")
                for g in range(NSG)
            ]
            v_g = [
                kv_p.tile([P, QW // P * D], bf16, tag=f"v{g}", name=f"v{g}")
                for g in range(NSG)
            ]
            qt_g = [
                kv_p.tile([P, QW], bf16, tag=f"qt{g}", name=f"qt{g}")
                for g in range(NQG)
            ]

            # ---- phase 1: xT chunks + float32r projections ----
            xr_tiles = {}
            for g in range(NSG):
                quarter = g // GPX
                if g % GPX == 0:
                    for ec in range(ECH):
                        if quarter == 0:
                            # half-width chunks: first projection starts sooner
                            subs = []
                            for h in range(GPX):
                                xh = xraw_p.tile(
                                    [P, QW], f32r, tag="xraw0",
                                    name=f"xr0_{ec}_{h}",
                                )
                                nc.sync.dma_start(
                                    xh[:],
                                    xt_d[
                                        ec * P : (ec + 1) * P,
                                        h * QW : (h + 1) * QW,
                                    ].bitcast(f32r),
                                )
                                subs.append(xh)
                            xr_tiles[(quarter, ec)] = subs
                        else:
                            xr = xraw_p.tile(
                                [P, XW], f32r, tag="xraw", name=f"xr{quarter}_{ec}"
                            )
                            nc.sync.dma_start(
                                xr[:],
                                xt_d[
                                    ec * P : (ec + 1) * P,
                                    quarter * XW : (quarter + 1) * XW,
                                ].bitcast(f32r),
                            )
                            xr_tiles[(quarter, ec)] = [xr]
                off = (g % GPX) * QW

                def rhs(ec):
                    tiles = xr_tiles[(quarter, ec)]
                    if len(tiles) > 1:
                        return tiles[g % GPX][:, 0:QW]
                    return tiles[0][:, off : off + QW]

                # K^T for this s-group
                pk = ps_proj.tile([P, QW], f32, tag="proj")
                for ec in range(ECH):
                    nc.tensor.matmul(
                        pk[:],
                        wk_sb[:, ec * D : (ec + 1) * D],
                        rhs(ec),
                        start=(ec == 0),
                        stop=(ec == ECH - 1),
                    )
                nc.vector.tensor_copy(kt_g[g][:], pk[:])
                # V^T then PE-transpose to V [s, d]
                pv = ps_proj.tile([P, QW], f32, tag="proj")
                for ec in range(ECH):
                    nc.tensor.matmul(
                        pv[:],
                        wv_sb[:, ec * D : (ec + 1) * D],
                        rhs(ec),
                        start=(ec == 0),
                        stop=(ec == ECH - 1),
                    )
                vt = vtsb_p.tile([P, QW], bf16, tag="vtsb")
                nc.vector.tensor_copy(vt[:], pv[:])
                for st in range(QW // P):
                    pvt = ps_small.tile([P, P], bf16, tag="proj")
                    nc.tensor.transpose(pvt[:], vt[:, st * P : (st + 1) * P], ident[:])
                    nc.vector.tensor_copy(v_g[g][:, st * D : (st + 1) * D], pvt[:])
                # Q^T only for the first H tokens
                if g < NQG:
                    pq = ps_proj.tile([P, QW], f32, tag="proj")
                    for ec in range(ECH):
                        nc.tensor.matmul(
                            pq[:],
                            wq_sb[:, ec * D : (ec + 1) * D],
                            rhs(ec),
                            start=(ec == 0),
                            stop=(ec == ECH - 1),
                        )
                    nc.vector.tensor_copy(qt_g[g][:], pq[:])

            # ---- phase 2: attention per q-group ----
            for g in range(NQG):
                kb_list = list(range(0, DIAG_PB * (g + 1))) + list(range(RB0, NKB))
                pav = ps_av.tile([P, QW], f32, tag="av")
                pden = ps_den.tile([1, QW], f32, tag="den")
                last = len(kb_list) - 1
                for i, kb in enumerate(kb_list):
                    sg, sb = kb // DIAG_PB, kb % DIAG_PB
                    pscore = ps_sc.tile([P, QW], f32, tag="sc")
                    nc.tensor.matmul(
                        pscore[:],
                        kt_g[sg][:, sb * KB : (sb + 1) * KB],
                        qt_g[g][:],
                        start=True,
                        stop=True,
                    )
                    pd = kb - DIAG_PB * g  # partial-diag index
                    if 0 <= pd < DIAG_PB:
                        nc.vector.tensor_add(
                            pscore[:],
                            pscore[:],
                            masks[:, pd * QW : (pd + 1) * QW],
                        )
                    et = expt_p.tile([P, QW], bf16, tag="expt")
                    if kb >= RB0:
                        bias = km_sb[:, kb - RB0 : kb - RB0 + 1]
                    else:
                        bias = 0.0
                    nc.scalar.activation(
                        et[:],
                        pscore[:],
                        mybir.ActivationFunctionType.Exp,
                        bias=bias,
                        scale=SCALE,
                    )
                    nc.tensor.matmul(
                        pav[:],
                        v_g[sg][:, sb * D : (sb + 1) * D],
                        et[:],
                        start=(i == 0),
                        stop=(i == last),
                    )
                    nc.tensor.matmul(
                        pden[:],
                        ones[:],
                        et[:],
                        start=(i == 0),
                        stop=(i == last),
                    )
                # epilogue: transpose unnormalized AV to [q, d]; fold the
                # 1/den into the post-transpose ACT copy (per-partition scale)
                recip = rec_p.tile([1, QW], f32, tag="recip")
                nc.vector.reciprocal(recip[:], pden[:])
                avn = avn_p.tile([P, QW], bf16, tag="avn")
                nc.vector.tensor_copy(avn[:], pav[:])
                osb = outsb_p.tile([P, QW // P * D], f32, tag="outsb")
                for qb in range(QW // P):
                    prc = ps_small.tile([P, 1], f32, tag="proj")
                    nc.tensor.transpose(
                        prc[:], recip[0:1, qb * P : (qb + 1) * P], identf[:]
                    )
                    rcol = rec_p.tile([P, 1], f32, tag="rcol")
                    nc.vector.tensor_copy(rcol[:], prc[:])
                    pout = ps_small.tile([P, P], bf16, tag="proj")
                    nc.tensor.transpose(
                        pout[:], avn[:, qb * P : (qb + 1) * P], ident[:]
                    )
                    nc.vector.tensor_scalar_mul(osb[:, qb * D : (qb + 1) * D], pout[:], rcol[:])
                nc.sync.dma_start(
                    out_d[g * QW : (g + 1) * QW, :].rearrange(
                        "(qb p) d -> p qb d", p=P
                    ),
                    osb[:].rearrange("p (qb d) -> p qb d", d=D),
                )

    nc.compile()
    return nc


_NC = None
LAST_RESULTS = None


def kernel(x, WQ, WK, WV):
    import os

    from concourse import bass_utils

    global _NC, LAST_RESULTS
    x = np.asarray(x, dtype=np.float32)
    WQ = np.ascontiguousarray(np.asarray(WQ, dtype=np.float32))
    WK = np.ascontiguousarray(np.asarray(WK, dtype=np.float32))
    WV = np.ascontiguousarray(np.asarray(WV, dtype=np.float32))

    if _NC is None:
        _NC = _build()
    nc = _NC

    in_maps = []
    for c in range(8):
        b, h = c >> 1, c & 1
        own = x[b, h * H : (h + 1) * H]
        other = x[b, (1 - h) * H : (2 - h) * H]
        # pool layout [own | other], transposed to [E, C] for the device
        xt_core = np.ascontiguousarray(np.concatenate([own, other], axis=0).T)
        km = np.full((P, (C - H) // KB), 0.0 if h == 1 else NEG, dtype=np.float32)
        in_maps.append({"xt": xt_core, "wq": WQ, "wk": WK, "wv": WV, "km": km})

    trace = os.environ.get("KERNEL_TRACE") == "1"
    res = bass_utils.run_bass_kernel_spmd(
        nc, in_maps, core_ids=list(range(8)), trace=trace
    )
    LAST_RESULTS = res

    out = np.empty((B, S, D), dtype=np.float32)
    for c in range(8):
        b, h = c >> 1, c & 1
        out[b, h * H : (h + 1) * H] = res.results[c]["out"]
    return out



# revision 4
# speedup vs baseline: 1.4498x; 1.4498x over previous
"""Causal single-head attention (B=4, S=4096, E=1024, D=128) on 8 TRN2 cores.

Sharding: core c = (batch b = c//2, key-parity p = c%2). Each core processes
ALL 4096 queries of its batch against the 2048 keys in the even (p=0) or odd
(p=1) 128-token key blocks, producing UNNORMALIZED partial attention
AV^T [D, S] and partial softmax denominators den [1, S]. The host combines
the two parity cores per batch: out = ((AV_e + AV_o) / (den_e + den_o))^T.
This makes every query group g attend a uniform prefix of 2(g+1) pool key
blocks on every core (72 blocks total vs 104 for the half-query sharding),
halves the K/V projection (no duplication across the pair), and needs no
collectives and no rectangle masking.

Within a pool prefix, the last 2 blocks straddle the causal diagonal; their
[128, 1024] staircase mask is per-core DATA (host-computed, parity-dependent)
added once per q-group on DVE. Everything else is unmasked.

All inputs are host-cast to bf16 (halves HBM traffic; TensorE runs bf16 at
1 cycle/row). x arrives pre-transposed per batch as xT [E, S]; K/V consume
parity-strided token slices via 3-dim moving APs, V is projected directly in
[s, d] layout (no PE transposes), and the AV partials are stored [d, q] and
transposed on the host. Scores are built transposed ([k, q]) so exp fuses on
ScalarE over [128, 1024] block-pairs and the AV/den matmuls consume exp
output directly. Softmax skips max-subtraction (score*scale stays ~[-5, 5]
for randn inputs)."""

import sys

if "/opt/trn_rl_repo" not in sys.path:
    sys.path.insert(0, "/opt/trn_rl_repo")

import numpy as np

B, S, E, D = 4, 4096, 1024, 128
SCALE = 1.0 / 32.0  # 1/sqrt(E)
NEG = -1.0e9
P = 128
QW = 512  # query group width
ECH = E // P  # 8 e-chunks
NQG = S // QW  # 8 query groups
NT = 4  # x chunks / kv pool groups (1024 tokens each)
XW = 1024


def _build():
    parity = 0  # odd-parity cores get host-swapped xt columns (see kernel())
    import concourse.bass as bass  # noqa: F401
    import concourse.mybir as mybir
    import concourse.tile as tile
    from concourse import bacc

    f32 = mybir.dt.float32
    bf16 = mybir.dt.bfloat16

    nc = bacc.Bacc(
        "TRN2",
        target_bir_lowering=False,
        debug=False,
        enable_asserts=False,
        num_devices=8,
    )

    xt_d = nc.dram_tensor("xt", [E, S], bf16, kind="ExternalInput").ap()
    wq_d = nc.dram_tensor("wq", [P, ECH * D], bf16, kind="ExternalInput").ap()
    wk_d = nc.dram_tensor("wk", [P, ECH * D], bf16, kind="ExternalInput").ap()
    wv_d = nc.dram_tensor("wv", [P, ECH * D], bf16, kind="ExternalInput").ap()
    mk_d = nc.dram_tensor("mk", [P, 2 * QW], bf16, kind="ExternalInput").ap()
    av_d = nc.dram_tensor("av", [P, S], f32, kind="ExternalOutput").ap()
    den_d = nc.dram_tensor("den", [1, S], f32, kind="ExternalOutput").ap()

    with tile.TileContext(nc) as tc:
        from contextlib import ExitStack

        with ExitStack() as ctx:
            consts = ctx.enter_context(tc.tile_pool(name="consts", bufs=1))
            xraw_p = ctx.enter_context(tc.tile_pool(name="xraw", bufs=16))
            kv_p = ctx.enter_context(tc.tile_pool(name="kv", bufs=1))
            expt_p = ctx.enter_context(tc.tile_pool(name="expt", bufs=4))
            avsb_p = ctx.enter_context(tc.tile_pool(name="avsb", bufs=2))
            ps_sc = ctx.enter_context(tc.tile_pool(name="ps_sc", bufs=2, space="PSUM"))
            ps_proj = ctx.enter_context(
                tc.tile_pool(name="ps_proj", bufs=2, space="PSUM")
            )
            ps_av = ctx.enter_context(tc.tile_pool(name="ps_av", bufs=1, space="PSUM"))
            ps_den = ctx.enter_context(
                tc.tile_pool(name="ps_den", bufs=1, space="PSUM")
            )

            # ---- weights / masks / constants ----
            w_sbufs = {}
            for name, w_d in (("wk", wk_d), ("wv", wv_d), ("wq", wq_d)):
                w_sb = consts.tile([P, ECH * D], bf16, tag=f"w_{name}", name=f"wsb_{name}")
                w_sbufs[name] = w_sb
                nc.scalar.dma_start(w_sb[:], w_d[:])
            wk_sb, wv_sb, wq_sb = w_sbufs["wk"], w_sbufs["wv"], w_sbufs["wq"]
            mk_sb = consts.tile([P, 2 * QW], bf16, tag="mk")
            nc.scalar.dma_start(mk_sb[:], mk_d[:])
            ones = consts.tile([P, 1], bf16, tag="ones")
            nc.gpsimd.memset(ones[:], 1.0)
            densb = consts.tile([1, S], f32, tag="densb")

            # per-pool-group projected tiles
            kt_g = [kv_p.tile([P, QW], bf16, tag=f"kt{t}", name=f"kt{t}") for t in range(NT)]
            v_g = [kv_p.tile([P, QW], bf16, tag=f"v{t}", name=f"v{t}") for t in range(NT)]
            qt_g = [kv_p.tile([P, QW], bf16, tag=f"qt{g}", name=f"qt{g}") for g in range(NQG)]

            xr = {}

            def load_chunk(t):
                for ec in range(ECH):
                    xh = xraw_p.tile([P, XW], bf16, tag="xraw", name=f"xr{t}_{ec}")
                    nc.sync.dma_start(
                        xh[:],
                        xt_d[ec * P : (ec + 1) * P, t * XW : (t + 1) * XW],
                    )
                    xr[(t, ec)] = xh

            load_chunk(0)

            for t in range(NT):
                if t + 1 < NT:
                    load_chunk(t + 1)

                # ---- K^T for pool group t: keys = parity-strided tokens ----
                pk = ps_proj.tile([P, QW], f32, tag="proj")
                for ec in range(ECH):
                    rhs3 = xr[(t, ec)][:].rearrange(
                        "p (f two h) -> p f two h", f=4, two=2
                    )[:, :, parity, :]
                    nc.tensor.matmul(
                        pk[:],
                        wk_sb[:, ec * D : (ec + 1) * D],
                        rhs3,
                        start=(ec == 0),
                        stop=(ec == ECH - 1),
                    )
                nc.vector.tensor_copy(kt_g[t][:], pk[:])

                # ---- V direct [s, d] for the 4 pool blocks of group t ----
                pv = ps_proj.tile([P, QW], f32, tag="proj")
                for mloc in range(4):
                    off = mloc * 256 + parity * P
                    for ec in range(ECH):
                        nc.tensor.matmul(
                            pv[:, mloc * P : (mloc + 1) * P],
                            xr[(t, ec)][:, off : off + P],
                            wv_sb[:, ec * D : (ec + 1) * D],
                            start=(ec == 0),
                            stop=(ec == ECH - 1),
                        )
                nc.vector.tensor_copy(v_g[t][:], pv[:])

                # ---- Q^T for query groups 2t, 2t+1 ----
                for g in (2 * t, 2 * t + 1):
                    pq = ps_proj.tile([P, QW], f32, tag="proj")
                    half = (g % 2) * QW
                    for ec in range(ECH):
                        nc.tensor.matmul(
                            pq[:],
                            wq_sb[:, ec * D : (ec + 1) * D],
                            xr[(t, ec)][:, half : half + QW],
                            start=(ec == 0),
                            stop=(ec == ECH - 1),
                        )
                    nc.vector.tensor_copy(qt_g[g][:], pq[:])

                # ---- attention for query groups 2t, 2t+1 ----
                for g in (2 * t, 2 * t + 1):
                    pav = ps_av.tile([P, QW], f32, tag="av")
                    pden = ps_den.tile([1, QW], f32, tag="den")
                    for pr in range(g + 1):
                        psc = ps_sc.tile([P, 2 * QW], f32, tag="sc")
                        for half in range(2):
                            m = 2 * pr + half  # pool block index
                            tk, ck = m // 4, (m % 4) * P
                            nc.tensor.matmul(
                                psc[:, half * QW : (half + 1) * QW],
                                kt_g[tk][:, ck : ck + P],
                                qt_g[g][:],
                                start=True,
                                stop=True,
                            )
                        if pr == g:
                            nc.vector.tensor_add(psc[:], psc[:], mk_sb[:])
                        et = expt_p.tile([P, 2 * QW], bf16, tag="expt")
                        nc.scalar.activation(
                            et[:],
                            psc[:],
                            mybir.ActivationFunctionType.Exp,
                            bias=0.0,
                            scale=SCALE,
                        )
                        for half in range(2):
                            m = 2 * pr + half
                            tk, ck = m // 4, (m % 4) * P
                            nc.tensor.matmul(
                                pav[:],
                                v_g[tk][:, ck : ck + P],
                                et[:, half * QW : (half + 1) * QW],
                                start=(pr == 0 and half == 0),
                                stop=(pr == g and half == 1),
                            )
                            nc.tensor.matmul(
                                pden[:],
                                ones[:],
                                et[:, half * QW : (half + 1) * QW],
                                start=(pr == 0 and half == 0),
                                stop=(pr == g and half == 1),
                            )
                    avsb = avsb_p.tile([P, QW], f32, tag="avsb")
                    nc.vector.tensor_copy(avsb[:], pav[:])
                    nc.sync.dma_start(av_d[:, g * QW : (g + 1) * QW], avsb[:])
                    nc.vector.tensor_copy(densb[:, g * QW : (g + 1) * QW], pden[:])

            nc.sync.dma_start(den_d[:], densb[:])

    nc.compile()
    return nc


_NC = None
LAST_RESULTS = None


def _masks(parity):
    """Mask for the diagonal block-pair of each query group, against the
    core's (possibly half-swapped) local query order."""
    import ml_dtypes

    mk = np.zeros((P, 2 * QW), dtype=np.float32)
    k = np.arange(P)[:, None]
    ql = np.arange(QW)[None, :]
    if parity:
        # local query ql maps to abs in-group offset with 128-halves of each
        # 256-span swapped
        sig = (ql // 256) * 256 + (1 - (ql % 256) // P) * P + (ql % P)
    else:
        sig = ql
    for dm in range(2):
        allowed = sig >= (k + 256 * dm + P * parity)
        mk[:, dm * QW : (dm + 1) * QW] = np.where(allowed, 0.0, NEG)
    return np.ascontiguousarray(mk.astype(ml_dtypes.bfloat16))


def kernel(x, WQ, WK, WV):
    import os

    import ml_dtypes

    from concourse import bass_utils

    global LAST_RESULTS, _NC
    bf = ml_dtypes.bfloat16
    x = np.asarray(x, dtype=np.float32)

    def prep_w(W):
        # [E, D] -> [P, ECH*D] with chunk ec at columns [ec*D, (ec+1)*D)
        W = np.asarray(W, dtype=np.float32)
        return np.ascontiguousarray(
            W.reshape(ECH, P, D).transpose(1, 0, 2).reshape(P, ECH * D)
        ).astype(bf)

    wq_h, wk_h, wv_h = prep_w(WQ), prep_w(WK), prep_w(WV)
    # column permutation: swap the two 128-halves of every 256-token span so
    # odd-parity cores see their key blocks in "slot 0" of each span
    swap = (
        (np.arange(S) // 256) * 256 + (1 - (np.arange(S) % 256) // P) * P
        + (np.arange(S) % P)
    )
    xt_nat = [np.ascontiguousarray(x[b].T.astype(bf)) for b in range(B)]
    xt_swp = [np.ascontiguousarray(xb[:, swap]) for xb in xt_nat]
    mk_all = [_masks(p) for p in range(2)]

    if _NC is None:
        _NC = _build()
    nc = _NC

    in_maps = []
    for c in range(8):
        b, p = c >> 1, c & 1
        in_maps.append(
            {
                "xt": xt_nat[b] if p == 0 else xt_swp[b],
                "wq": wq_h,
                "wk": wk_h,
                "wv": wv_h,
                "mk": mk_all[p],
            }
        )

    trace = os.environ.get("KERNEL_TRACE") == "1"
    res = bass_utils.run_bass_kernel_spmd(
        nc, in_maps, core_ids=list(range(8)), trace=trace
    )
    LAST_RESULTS = res

    out = np.empty((B, S, D), dtype=np.float32)
    for b in range(B):
        av0, den0 = res.results[2 * b]["av"], res.results[2 * b]["den"]
        av1, den1 = res.results[2 * b + 1]["av"], res.results[2 * b + 1]["den"]
        # odd-parity core's query columns are half-swapped: undo
        av = av0 + av1[:, swap]
        den = den0 + den1[:, swap]
        out[b] = (av / den).T
    return out
